# revision 17
# baseline (speedup 1.0000x reference)
"""HQQ 4-bit quantized linear layer on 8 Trainium2 NeuronCores.

Reference computation:
    W_r = concat([W_q >> 4, W_q & 0xF], axis=0).astype(f32)    # [64, 704512]
    W   = ((W_r - zero) * scale).reshape(11008, 4096)          # [out, in]
    out = x @ W.T + bias                                        # [4, 2048, 11008]

Group structure: group j = r*4096 + k (r in [0,172), k in [0,4096)) supplies
output feature o = i*172 + r (element i in [0,64) of the group) at input
feature k.  So for a fixed o, every k belongs to a different group, and
W[o, k] = (nib[i, j] - zero[j]) * scale[j] with i = o//172, j = (o%172)*4096+k.

Sharding (column-parallel over output features, SPMD-uniform):
  core c owns W_q byte-rows [4c, 4c+4).  Byte-row b holds the high nibble of
  i=b and the low nibble of i=b+32, so core c produces output features
  o in {(4c+ib)*172 + r} (high) and {(32+4c+ib)*172 + r} (low), ib in [0,4),
  r in [0,172): 1376 contiguous-after-gather features per core.  The host
  decodes the packed-4bit storage format into per-core nibble planes
  (exact small ints, shipped as u8); x / scale / zero are replicated.

Device kernel (per core), tuned from trace analysis:
  - Dequant arithmetic runs on DVE as pure-bf16 tensor_tensor ops (2x DVE
    tier): w = (nib - zero_b) * scale_b; nibble/scale/zero tiles are cast to
    bf16 on the otherwise-idle ScalarE.  ~1.75us of DVE per k-tile instead of
    the 4.5us the packed-u8 device-unpack path took, and u8 nibble planes
    keep the prologue weight DMA at 1.4MB per k-tile pair (it was DMA-bound
    at bf16 width).
  - The first 256-token superstep is emitted kt-major across both its PSUM
    sets so the PE lag-follows the dequant tile-by-tile instead of stalling
    (the old version spent 130us at half HAM clock behind the dequant).
    The two spare PSUM banks hold superstep-1's first 512-col chunk, giving
    the PE extra work during the dequant window.
  - Remaining supersteps stream x^T tiles (f32->bf16 cast on ScalarE) and
    matmul-accumulate 32 k-tiles into PSUM, drain + bias on VectorE.
"""

import os
import sys

for _p in ("/opt/trn_rl_repo",):
    if os.path.isdir(_p) and _p not in sys.path:
        sys.path.insert(0, _p)

import numpy as np

P = 128
IN_F = 4096
OUT_F = 11008
GROUP = 64
R_FULL = 172          # OUT_F // GROUP
IB_FULL = 4           # W_q byte rows per core
N_CORES = 8
NTOK_FULL = 8192      # 4 * 2048


def _chunks(n, maxc=512):
    out = []
    off = 0
    while off < n:
        sz = min(maxc, n - off)
        out.append((off, sz))
        off += sz
    return out


def build_program(KT=32, NSUP=32, SUP=256, IB=IB_FULL, R=R_FULL, num_devices=N_CORES):
    """Build the SPMD bass program. Returns the compiled Bacc object.

    KT: number of 128-wide k tiles (K = 128*KT)
    NSUP: number of token supersteps;  SUP: tokens per superstep (mult of 128)
    IB: W_q byte rows per core;  R: group minor dim (o = i*R + r)
    """
    import concourse.bacc as bacc
    import concourse.bass as bass
    import concourse.mybir as mybir
    import concourse.tile as tile
    from concourse.alu_op_type import AluOpType

    f32 = mybir.dt.float32
    bf16 = mybir.dt.bfloat16
    u8 = mybir.dt.uint8

    K = KT * P
    NTOK = NSUP * SUP
    NSUB = SUP // P
    OHALF = IB * R
    OFULL = 2 * OHALF
    CHUNKS = _chunks(OFULL)

    nc = bacc.Bacc(
        "TRN2", target_bir_lowering=False, debug=False, num_devices=num_devices
    )

    xt = nc.dram_tensor("xt", [K, NTOK], f32, kind="ExternalInput")
    # host-decoded nibble planes (integers 0..15 as u8): [K, 2*OHALF]
    wn = nc.dram_tensor("wn", [K, OFULL], u8, kind="ExternalInput")
    scale_t = nc.dram_tensor("scale_t", [K, R], f32, kind="ExternalInput")
    zero_t = nc.dram_tensor("zero_t", [K, R], f32, kind="ExternalInput")
    bias = nc.dram_tensor("bias", [OFULL], f32, kind="ExternalInput")
    out = nc.dram_tensor("out", [NTOK, OFULL], f32, kind="ExternalOutput")

    with tile.TileContext(nc) as tc:
        with (
            tc.tile_pool(name="cst", bufs=1) as cst,
            tc.tile_pool(name="wres", bufs=1) as wres,
            tc.tile_pool(name="dq", bufs=4) as dq,
            tc.tile_pool(name="xload", bufs=6) as xp,
            tc.tile_pool(name="xb", bufs=2) as xbp,
            tc.tile_pool(name="psum", bufs=2, space="PSUM") as pp,
            tc.tile_pool(name="psx", bufs=1, space="PSUM") as ppx,
            tc.tile_pool(name="outp", bufs=3) as op,
        ):
            w_res = [
                wres.tile([P, OFULL], bf16, tag=f"w{kt}", name=f"w{kt}")
                for kt in range(KT)
            ]

            # --- dequant (DVE) + first-superstep x staging, interleaved per
            # k-tile so weight DMAs lead and the PE can follow tile-by-tile ---
            xb0 = xbp.tile([P, KT, SUP], bf16, tag="xb")
            for kt in range(KT):
                ks = slice(kt * P, (kt + 1) * P)
                wn_t = dq.tile([P, OFULL], u8, tag="wn")
                nc.sync.dma_start(out=wn_t[:], in_=wn[ks, :])
                sc = dq.tile([P, R], f32, tag="sc")
                nc.sync.dma_start(out=sc[:], in_=scale_t[ks, :])
                zr = dq.tile([P, R], f32, tag="zr")
                nc.sync.dma_start(out=zr[:], in_=zero_t[ks, :])
                # scale/zero bf16 casts are cheap 2x-tier DVE copies; the big
                # u8 nibble cast stays on ScalarE (ACT and DVE each ~2us/ktile)
                scb = dq.tile([P, R], bf16, tag="scb")
                nc.vector.tensor_scalar(
                    out=scb[:], in0=sc[:], scalar1=1.0, scalar2=None,
                    op0=AluOpType.mult,
                )
                zrb = dq.tile([P, R], bf16, tag="zrb")
                nc.vector.tensor_scalar(
                    out=zrb[:], in0=zr[:], scalar1=1.0, scalar2=None,
                    op0=AluOpType.mult,
                )
                wnb = dq.tile([P, OFULL], bf16, tag="wnb")
                nc.scalar.copy(out=wnb[:], in_=wn_t[:])
                # broadcast [P, R] -> [P, 2*IB, R] with stride-0 middle dim
                scb_b = bass.AP(scb.tensor, scb.offset, [scb.ap[0], [0, 2 * IB], [1, R]])
                zrb_b = bass.AP(zrb.tensor, zrb.offset, [zrb.ap[0], [0, 2 * IB], [1, R]])

                nib2 = dq.tile([P, OFULL], bf16, tag="nib2", name="nib2")
                nc.vector.tensor_tensor(
                    out=nib2[:], in0=wnb[:], in1=zrb_b, op=AluOpType.subtract
                )
                nc.vector.tensor_tensor(
                    out=w_res[kt][:], in0=nib2[:], in1=scb_b, op=AluOpType.mult
                )

                # superstep-0 x staging for this k-tile
                xf = xp.tile([P, SUP], f32, tag="xf")
                nc.sync.dma_start(out=xf[:], in_=xt[ks, 0:SUP])
                nc.scalar.copy(out=xb0[:, kt, :], in_=xf[:])

            # bias broadcast to [128, OFULL] via partition-step-0 DMA read
            bias_b = cst.tile([P, OFULL], f32)
            bias_bcast_src = bass.AP(bias, 0, [[0, P], [1, OFULL]])
            nc.sync.dma_start(out=bias_b[:], in_=bias_bcast_src)

            # --- superstep 0: kt-major over both PSUM sets (lag-follows
            # dequant); spare PSUM banks pre-compute superstep 1's chunk 0 ---
            # superstep-1 x staging is needed during the dequant window (the
            # spare-bank chunk-0 matmuls consume it), so its casts run on the
            # otherwise-idle GpSimd engine instead of loading ACT further
            xb1 = xbp.tile([P, KT, SUP], bf16, tag="xb")
            for kt in range(KT):
                xf = xp.tile([P, SUP], f32, tag="xf")
                nc.sync.dma_start(
                    out=xf[:], in_=xt[kt * P:(kt + 1) * P, SUP:2 * SUP]
                )
                nc.gpsimd.tensor_scalar(
                    out=xb1[:, kt, :], in0=xf[:], scalar1=1.0, scalar2=None,
                    op0=AluOpType.mult,
                )

            ps0 = [
                [
                    pp.tile([P, sz], f32, tag=f"ps{ci}", name=f"ps{ci}")
                    for ci, (off, sz) in enumerate(CHUNKS)
                ]
                for _ in range(NSUB)
            ]
            c0_off, c0_sz = CHUNKS[0]
            psx = [
                ppx.tile([P, c0_sz], f32, tag=f"px{sub}", name=f"px{sub}")
                for sub in range(NSUB)
            ]
            for kt in range(KT):
                st = (kt == 0)
                sp = (kt == KT - 1)
                for sub in range(NSUB):
                    lhsT = xb0[:, kt, sub * P:(sub + 1) * P]
                    for ci, (off, sz) in enumerate(CHUNKS):
                        nc.tensor.matmul(
                            ps0[sub][ci][:], lhsT, w_res[kt][:, off:off + sz],
                            start=st, stop=sp,
                        )
                for sub in range(NSUB):
                    nc.tensor.matmul(
                        psx[sub][:],
                        xb1[:, kt, sub * P:(sub + 1) * P],
                        w_res[kt][:, c0_off:c0_off + c0_sz],
                        start=st, stop=sp,
                    )
            for sub in range(NSUB):
                ot = op.tile([P, OFULL], f32, tag="ot")
                row0 = sub * P
                for ci, (off, sz) in enumerate(CHUNKS):
                    nc.vector.tensor_add(
                        out=ot[:, off:off + sz], in0=ps0[sub][ci][:],
                        in1=bias_b[:, off:off + sz],
                    )
                    nc.sync.dma_start(
                        out=out[row0:row0 + P, off:off + sz],
                        in_=ot[:, off:off + sz],
                    )

            # --- steady-state supersteps (ns=1 reuses the pre-computed
            # chunk-0 partials from the prologue) ---
            for ns in range(1, NSUP):
                tok0 = ns * SUP
                if ns == 1:
                    xb = xb1
                else:
                    xb = xbp.tile([P, KT, SUP], bf16, tag="xb")
                    for kt in range(KT):
                        xf = xp.tile([P, SUP], f32, tag="xf")
                        nc.sync.dma_start(
                            out=xf[:], in_=xt[kt * P:(kt + 1) * P, tok0:tok0 + SUP]
                        )
                        # f32 -> bf16 cast on ScalarE (ACT otherwise idle)
                        nc.scalar.copy(out=xb[:, kt, :], in_=xf[:])
                for sub in range(NSUB):
                    chunks_here = list(enumerate(CHUNKS))
                    ps = {}
                    for ci, (off, sz) in chunks_here:
                        if ns == 1 and ci == 0:
                            ps[ci] = psx[sub]
                        else:
                            ps[ci] = pp.tile([P, sz], f32, tag=f"ps{ci}", name=f"ps{ci}")
                    for kt in range(KT):
                        lhsT = xb[:, kt, sub * P:(sub + 1) * P]
                        for ci, (off, sz) in chunks_here:
                            if ns == 1 and ci == 0:
                                continue  # chunk 0 accumulated in the prologue
                            nc.tensor.matmul(
                                ps[ci][:], lhsT, w_res[kt][:, off:off + sz],
                                start=(kt == 0), stop=(kt == KT - 1),
                            )
                    ot = op.tile([P, OFULL], f32, tag="ot")
                    row0 = tok0 + sub * P
                    for ci, (off, sz) in chunks_here:
                        nc.vector.tensor_add(
                            out=ot[:, off:off + sz], in0=ps[ci][:],
                            in1=bias_b[:, off:off + sz],
                        )
                        nc.sync.dma_start(
                            out=out[row0:row0 + P, off:off + sz],
                            in_=ot[:, off:off + sz],
                        )

    nc.compile()
    return nc


_PROG_CACHE = {}


def _get_program():
    key = "full"
    if key not in _PROG_CACHE:
        _PROG_CACHE[key] = build_program()
    return _PROG_CACHE[key]


def shard_inputs(x, W_q, scale, zero, bias):
    """Host-side sharding / layout transforms.

    Only lossless layout work happens here: transposes, the packed-4bit
    storage-format decode (two nibble planes per byte row), and exact
    re-encoding of the 4-bit integers as bf16.  All value arithmetic
    (zero/scale dequant, matmul, bias) runs on device.
    """
    x = np.asarray(x, dtype=np.float32)
    W_q = np.asarray(W_q)
    scale = np.asarray(scale, dtype=np.float32)
    zero = np.asarray(zero, dtype=np.float32)
    bias = np.asarray(bias, dtype=np.float32)

    ntok = x.shape[0] * x.shape[1]
    xt = np.ascontiguousarray(x.reshape(ntok, IN_F).T)              # [K, NTOK]
    scale_t = np.ascontiguousarray(scale.reshape(R_FULL, IN_F).T)   # [K, R]
    zero_t = np.ascontiguousarray(zero.reshape(R_FULL, IN_F).T)     # [K, R]
    wq_u8 = W_q.astype(np.uint8)                                    # values < 256
    bias_rs = bias.reshape(GROUP, R_FULL)                           # [i, r]

    in_maps = []
    for c in range(N_CORES):
        rows = wq_u8[IB_FULL * c: IB_FULL * (c + 1)]                # [4, 704512]
        # packed-4bit decode: high nibble -> feature i=b, low -> i=b+32
        nib = np.stack([rows >> 4, rows & 0xF])                     # [2, 4, 704512]
        # [half, ib, r, k] -> [k, half, ib, r] -> [K, OFULL]
        wn_c = np.ascontiguousarray(
            nib.reshape(2, IB_FULL, R_FULL, IN_F).transpose(3, 0, 1, 2)
        ).reshape(IN_F, 2 * IB_FULL * R_FULL)
        bias_c = np.concatenate(
            [
                bias_rs[IB_FULL * c: IB_FULL * (c + 1)].ravel(),
                bias_rs[32 + IB_FULL * c: 32 + IB_FULL * (c + 1)].ravel(),
            ]
        )
        in_maps.append(
            {
                "xt": xt,
                "wn": wn_c,
                "scale_t": scale_t,
                "zero_t": zero_t,
                "bias": bias_c,
            }
        )
    return in_maps


def gather_output(results, ntok=NTOK_FULL):
    out = np.empty((ntok, OUT_F), dtype=np.float32)
    ohalf = IB_FULL * R_FULL
    for c in range(N_CORES):
        res = results[c]["out"]
        lo = IB_FULL * c * R_FULL
        out[:, lo: lo + ohalf] = res[:, :ohalf]
        lo = (32 + IB_FULL * c) * R_FULL
        out[:, lo: lo + ohalf] = res[:, ohalf:]
    return out


def kernel(x, W_q, scale, zero, bias):
    from concourse.bass_utils import run_bass_kernel_spmd

    x = np.asarray(x)
    b, s, _ = x.shape
    nc = _get_program()
    in_maps = shard_inputs(x, W_q, scale, zero, bias)
    res = run_bass_kernel_spmd(nc, in_maps, list(range(N_CORES)))
    out = gather_output(res.results)
    return out.reshape(b, s, OUT_F)


# revision 21
# speedup vs baseline: 1.0783x; 1.0783x over previous
"""HQQ 4-bit quantized linear layer on 8 Trainium2 NeuronCores.

Reference computation:
    W_r = concat([W_q >> 4, W_q & 0xF], axis=0).astype(f32)    # [64, 704512]
    W   = ((W_r - zero) * scale).reshape(11008, 4096)          # [out, in]
    out = x @ W.T + bias                                        # [4, 2048, 11008]

Group structure: group j = r*4096 + k (r in [0,172), k in [0,4096)) supplies
output feature o = i*172 + r (element i in [0,64) of the group) at input
feature k.  So for a fixed o, every k belongs to a different group, and
W[o, k] = (nib[i, j] - zero[j]) * scale[j] with i = o//172, j = (o%172)*4096+k.

Sharding (column-parallel over output features, SPMD-uniform):
  core c owns W_q byte-rows [4c, 4c+4).  Byte-row b holds the high nibble of
  i=b and the low nibble of i=b+32, so core c produces output features
  o in {(4c+ib)*172 + r} (high) and {(32+4c+ib)*172 + r} (low), ib in [0,4),
  r in [0,172): 1376 contiguous-after-gather features per core.  The host
  decodes the packed-4bit storage format into per-core nibble planes
  (exact small ints, shipped as u8); x / scale / zero are replicated.

Device kernel (per core), tuned from trace analysis:
  - Dequant arithmetic runs on DVE as pure-bf16 tensor_tensor ops (2x DVE
    tier): w = (nib - zero_b) * scale_b; nibble/scale/zero tiles are cast to
    bf16 on the otherwise-idle ScalarE.  ~1.75us of DVE per k-tile instead of
    the 4.5us the packed-u8 device-unpack path took, and u8 nibble planes
    keep the prologue weight DMA at 1.4MB per k-tile pair (it was DMA-bound
    at bf16 width).
  - The first 256-token superstep is emitted kt-major across both its PSUM
    sets so the PE lag-follows the dequant tile-by-tile instead of stalling
    (the old version spent 130us at half HAM clock behind the dequant).
    The two spare PSUM banks hold superstep-1's first 512-col chunk, giving
    the PE extra work during the dequant window.
  - Remaining supersteps stream x^T tiles (f32->bf16 cast on ScalarE) and
    matmul-accumulate 32 k-tiles into PSUM, drain + bias on VectorE.
"""

import os
import sys

for _p in ("/opt/trn_rl_repo",):
    if os.path.isdir(_p) and _p not in sys.path:
        sys.path.insert(0, _p)

import numpy as np

P = 128
IN_F = 4096
OUT_F = 11008
GROUP = 64
R_FULL = 172          # OUT_F // GROUP
IB_FULL = 4           # W_q byte rows per core
N_CORES = 8
NTOK_FULL = 8192      # 4 * 2048


def _chunks(n, maxc=512):
    out = []
    off = 0
    while off < n:
        sz = min(maxc, n - off)
        out.append((off, sz))
        off += sz
    return out


def build_program(KT=32, NSUP=32, SUP=256, IB=IB_FULL, R=R_FULL, num_devices=N_CORES):
    """Build the SPMD bass program. Returns the compiled Bacc object.

    KT: number of 128-wide k tiles (K = 128*KT)
    NSUP: number of token supersteps;  SUP: tokens per superstep (mult of 128)
    IB: W_q byte rows per core;  R: group minor dim (o = i*R + r)
    """
    import concourse.bacc as bacc
    import concourse.bass as bass
    import concourse.mybir as mybir
    import concourse.tile as tile
    from concourse.alu_op_type import AluOpType

    f32 = mybir.dt.float32
    bf16 = mybir.dt.bfloat16
    u8 = mybir.dt.uint8

    K = KT * P
    NTOK = NSUP * SUP
    NSUB = SUP // P
    OHALF = IB * R
    OFULL = 2 * OHALF
    CHUNKS = _chunks(OFULL)

    nc = bacc.Bacc(
        "TRN2", target_bir_lowering=False, debug=False, num_devices=num_devices
    )

    xt = nc.dram_tensor("xt", [K, NTOK], f32, kind="ExternalInput")
    # host-decoded nibble planes (integers 0..15 as u8): [K, 2*OHALF]
    wn = nc.dram_tensor("wn", [K, OFULL], u8, kind="ExternalInput")
    scale_t = nc.dram_tensor("scale_t", [K, R], f32, kind="ExternalInput")
    zero_t = nc.dram_tensor("zero_t", [K, R], f32, kind="ExternalInput")
    bias = nc.dram_tensor("bias", [OFULL], f32, kind="ExternalInput")
    out = nc.dram_tensor("out", [NTOK, OFULL], f32, kind="ExternalOutput")

    with tile.TileContext(nc) as tc:
        with (
            tc.tile_pool(name="cst", bufs=1) as cst,
            tc.tile_pool(name="wres", bufs=1) as wres,
            tc.tile_pool(name="dq", bufs=4) as dq,
            tc.tile_pool(name="xload", bufs=6) as xp,
            tc.tile_pool(name="xb", bufs=2) as xbp,
            tc.tile_pool(name="psum", bufs=2, space="PSUM") as pp,
            tc.tile_pool(name="psx", bufs=1, space="PSUM") as ppx,
            tc.tile_pool(name="outp", bufs=3) as op,
        ):
            w_res = [
                wres.tile([P, OFULL], bf16, tag=f"w{kt}", name=f"w{kt}")
                for kt in range(KT)
            ]

            # --- dequant (DVE) + first-superstep x staging, interleaved per
            # k-tile so weight DMAs lead and the PE can follow tile-by-tile ---
            xb0 = xbp.tile([P, KT, SUP], bf16, tag="xb")
            xb1 = xbp.tile([P, KT, SUP], bf16, tag="xb")
            for kt in range(KT):
                ks = slice(kt * P, (kt + 1) * P)
                wn_t = dq.tile([P, OFULL], u8, tag="wn")
                nc.sync.dma_start(out=wn_t[:], in_=wn[ks, :])
                sc = dq.tile([P, R], f32, tag="sc")
                nc.sync.dma_start(out=sc[:], in_=scale_t[ks, :])
                zr = dq.tile([P, R], f32, tag="zr")
                nc.sync.dma_start(out=zr[:], in_=zero_t[ks, :])
                # scale/zero bf16 casts are cheap 2x-tier DVE copies; the big
                # u8 nibble cast stays on ScalarE (ACT and DVE each ~2us/ktile)
                scb = dq.tile([P, R], bf16, tag="scb")
                nc.vector.tensor_scalar(
                    out=scb[:], in0=sc[:], scalar1=1.0, scalar2=None,
                    op0=AluOpType.mult,
                )
                zrb = dq.tile([P, R], bf16, tag="zrb")
                nc.vector.tensor_scalar(
                    out=zrb[:], in0=zr[:], scalar1=1.0, scalar2=None,
                    op0=AluOpType.mult,
                )
                wnb = dq.tile([P, OFULL], bf16, tag="wnb")
                nc.scalar.copy(out=wnb[:], in_=wn_t[:])
                # broadcast [P, R] -> [P, 2*IB, R] with stride-0 middle dim
                scb_b = bass.AP(scb.tensor, scb.offset, [scb.ap[0], [0, 2 * IB], [1, R]])
                zrb_b = bass.AP(zrb.tensor, zrb.offset, [zrb.ap[0], [0, 2 * IB], [1, R]])

                nib2 = dq.tile([P, OFULL], bf16, tag="nib2", name="nib2")
                nc.vector.tensor_tensor(
                    out=nib2[:], in0=wnb[:], in1=zrb_b, op=AluOpType.subtract
                )
                nc.vector.tensor_tensor(
                    out=w_res[kt][:], in0=nib2[:], in1=scb_b, op=AluOpType.mult
                )

                # superstep-0/1 x staging for this k-tile (both consumed by
                # the kt-major prologue matmuls below); superstep-1's cast
                # rides on DVE so ACT keeps pacing the nibble casts
                # (GpSimd was tried for it: 3.8us per cast, dispatch-bound)
                xf = xp.tile([P, SUP], f32, tag="xf")
                nc.sync.dma_start(out=xf[:], in_=xt[ks, 0:SUP])
                nc.scalar.copy(out=xb0[:, kt, :], in_=xf[:])
                xf1 = xp.tile([P, SUP], f32, tag="xf")
                nc.sync.dma_start(out=xf1[:], in_=xt[ks, SUP:2 * SUP])
                nc.vector.tensor_scalar(
                    out=xb1[:, kt, :], in0=xf1[:], scalar1=1.0, scalar2=None,
                    op0=AluOpType.mult,
                )

            # bias broadcast to [128, OFULL] via partition-step-0 DMA read
            bias_b = cst.tile([P, OFULL], f32)
            bias_bcast_src = bass.AP(bias, 0, [[0, P], [1, OFULL]])
            nc.sync.dma_start(out=bias_b[:], in_=bias_bcast_src)

            # --- superstep 0: kt-major over both PSUM sets (lag-follows
            # dequant); spare PSUM banks pre-compute superstep 1's chunk 0 ---
            ps0 = [
                [
                    pp.tile([P, sz], f32, tag=f"ps{ci}", name=f"ps{ci}")
                    for ci, (off, sz) in enumerate(CHUNKS)
                ]
                for _ in range(NSUB)
            ]
            c0_off, c0_sz = CHUNKS[0]
            psx = [
                ppx.tile([P, c0_sz], f32, tag=f"px{sub}", name=f"px{sub}")
                for sub in range(NSUB)
            ]
            for kt in range(KT):
                st = (kt == 0)
                sp = (kt == KT - 1)
                for sub in range(NSUB):
                    lhsT = xb0[:, kt, sub * P:(sub + 1) * P]
                    for ci, (off, sz) in enumerate(CHUNKS):
                        nc.tensor.matmul(
                            ps0[sub][ci][:], lhsT, w_res[kt][:, off:off + sz],
                            start=st, stop=sp,
                        )
                for sub in range(NSUB):
                    nc.tensor.matmul(
                        psx[sub][:],
                        xb1[:, kt, sub * P:(sub + 1) * P],
                        w_res[kt][:, c0_off:c0_off + c0_sz],
                        start=st, stop=sp,
                    )
            for sub in range(NSUB):
                ot = op.tile([P, OFULL], f32, tag="ot")
                row0 = sub * P
                for ci, (off, sz) in enumerate(CHUNKS):
                    nc.vector.tensor_add(
                        out=ot[:, off:off + sz], in0=ps0[sub][ci][:],
                        in1=bias_b[:, off:off + sz],
                    )
                    nc.sync.dma_start(
                        out=out[row0:row0 + P, off:off + sz],
                        in_=ot[:, off:off + sz],
                    )

            # --- steady-state supersteps (ns=1 reuses the pre-computed
            # chunk-0 partials from the prologue) ---
            for ns in range(1, NSUP):
                tok0 = ns * SUP
                if ns == 1:
                    xb = xb1
                else:
                    xb = xbp.tile([P, KT, SUP], bf16, tag="xb")
                    for kt in range(KT):
                        xf = xp.tile([P, SUP], f32, tag="xf")
                        nc.sync.dma_start(
                            out=xf[:], in_=xt[kt * P:(kt + 1) * P, tok0:tok0 + SUP]
                        )
                        # f32 -> bf16 cast on ScalarE (ACT otherwise idle)
                        nc.scalar.copy(out=xb[:, kt, :], in_=xf[:])
                for sub in range(NSUB):
                    chunks_here = list(enumerate(CHUNKS))
                    ps = {}
                    for ci, (off, sz) in chunks_here:
                        if ns == 1 and ci == 0:
                            ps[ci] = psx[sub]
                        else:
                            ps[ci] = pp.tile([P, sz], f32, tag=f"ps{ci}", name=f"ps{ci}")
                    for kt in range(KT):
                        lhsT = xb[:, kt, sub * P:(sub + 1) * P]
                        for ci, (off, sz) in chunks_here:
                            if ns == 1 and ci == 0:
                                continue  # chunk 0 accumulated in the prologue
                            nc.tensor.matmul(
                                ps[ci][:], lhsT, w_res[kt][:, off:off + sz],
                                start=(kt == 0), stop=(kt == KT - 1),
                            )
                    ot = op.tile([P, OFULL], f32, tag="ot")
                    row0 = tok0 + sub * P
                    for ci, (off, sz) in chunks_here:
                        nc.vector.tensor_add(
                            out=ot[:, off:off + sz], in0=ps[ci][:],
                            in1=bias_b[:, off:off + sz],
                        )
                        nc.sync.dma_start(
                            out=out[row0:row0 + P, off:off + sz],
                            in_=ot[:, off:off + sz],
                        )

    nc.compile()
    return nc


_PROG_CACHE = {}


def _get_program():
    key = "full"
    if key not in _PROG_CACHE:
        _PROG_CACHE[key] = build_program()
    return _PROG_CACHE[key]


def shard_inputs(x, W_q, scale, zero, bias):
    """Host-side sharding / layout transforms.

    Only lossless layout work happens here: transposes, the packed-4bit
    storage-format decode (two nibble planes per byte row), and exact
    re-encoding of the 4-bit integers as bf16.  All value arithmetic
    (zero/scale dequant, matmul, bias) runs on device.
    """
    x = np.asarray(x, dtype=np.float32)
    W_q = np.asarray(W_q)
    scale = np.asarray(scale, dtype=np.float32)
    zero = np.asarray(zero, dtype=np.float32)
    bias = np.asarray(bias, dtype=np.float32)

    ntok = x.shape[0] * x.shape[1]
    xt = np.ascontiguousarray(x.reshape(ntok, IN_F).T)              # [K, NTOK]
    scale_t = np.ascontiguousarray(scale.reshape(R_FULL, IN_F).T)   # [K, R]
    zero_t = np.ascontiguousarray(zero.reshape(R_FULL, IN_F).T)     # [K, R]
    wq_u8 = W_q.astype(np.uint8)                                    # values < 256
    bias_rs = bias.reshape(GROUP, R_FULL)                           # [i, r]

    in_maps = []
    for c in range(N_CORES):
        rows = wq_u8[IB_FULL * c: IB_FULL * (c + 1)]                # [4, 704512]
        # packed-4bit decode: high nibble -> feature i=b, low -> i=b+32
        nib = np.stack([rows >> 4, rows & 0xF])                     # [2, 4, 704512]
        # [half, ib, r, k] -> [k, half, ib, r] -> [K, OFULL]
        wn_c = np.ascontiguousarray(
            nib.reshape(2, IB_FULL, R_FULL, IN_F).transpose(3, 0, 1, 2)
        ).reshape(IN_F, 2 * IB_FULL * R_FULL)
        bias_c = np.concatenate(
            [
                bias_rs[IB_FULL * c: IB_FULL * (c + 1)].ravel(),
                bias_rs[32 + IB_FULL * c: 32 + IB_FULL * (c + 1)].ravel(),
            ]
        )
        in_maps.append(
            {
                "xt": xt,
                "wn": wn_c,
                "scale_t": scale_t,
                "zero_t": zero_t,
                "bias": bias_c,
            }
        )
    return in_maps


def gather_output(results, ntok=NTOK_FULL):
    out = np.empty((ntok, OUT_F), dtype=np.float32)
    ohalf = IB_FULL * R_FULL
    for c in range(N_CORES):
        res = results[c]["out"]
        lo = IB_FULL * c * R_FULL
        out[:, lo: lo + ohalf] = res[:, :ohalf]
        lo = (32 + IB_FULL * c) * R_FULL
        out[:, lo: lo + ohalf] = res[:, ohalf:]
    return out


def kernel(x, W_q, scale, zero, bias):
    from concourse.bass_utils import run_bass_kernel_spmd

    x = np.asarray(x)
    b, s, _ = x.shape
    nc = _get_program()
    in_maps = shard_inputs(x, W_q, scale, zero, bias)
    res = run_bass_kernel_spmd(nc, in_maps, list(range(N_CORES)))
    out = gather_output(res.results)
    return out.reshape(b, s, OUT_F)
